# revision 1
# baseline (speedup 1.0000x reference)
# Trainium2 Bass kernel for nn_CausalSelfAttention_13022340841799.
#
# Problem (hardcoded shapes): B=2, L=4096, D=512, 8 heads of dim 64.
#   qkv = x @ w_in + b_in; prefix-causal attention (PREFIX=1: tril mask with
#   column 0 disallowed for rows >= 1); out = attn_out @ w_out + b_out.
#
# Sharding: 8 cores = 2 batches x 4 head-pairs. Core c handles batch c//4 and
# heads {2*(c%4), 2*(c%4)+1}. Each core computes a partial [L, D] output
# (its heads' contribution through w_out); the host sums the 4 partials per
# batch and adds b_out.
#
# Device algorithm (per core), flash-attention style in "transposed" layout:
#   xT [D, L] built via PE transposes; qT/kT = (w.T @ xT) [128, L];
#   v natural [L, 128]. Per head: S^T tiles [128 cols, 512 rows] = k_j^T q_r,
#   P^T = exp(S^T/8) * mask (bf16), O_aug^T [65, 512] += v_aug_j^T @ P^T where
#   v_aug has a ones column => row 64 accumulates the softmax denominator.
#   Normalize with DVE reciprocal + gpsimd partition broadcast, then
#   out partial = sum_h O_h @ w_out[h rows] in PSUM, DMA to DRAM.
# Compute dtype bf16 (f32 PSUM accumulate); masks/causal-skip halve the work.

import numpy as np

import concourse.bass as bass
import concourse.mybir as mybir
import concourse.tile as tile
from concourse import bacc
from concourse.bass_utils import run_bass_kernel_spmd
from concourse.masks import make_identity

F32 = mybir.dt.float32
BF16 = mybir.dt.bfloat16

B, L, D = 2, 4096, 512
H, HD = 8, 64
HPC = 2                  # heads per core
CD = HPC * HD            # 128 per-core qkv feature columns
NCORES = 8
SCALE = 1.0 / 8.0        # 1/sqrt(64)
RB = L // 128            # 32 row blocks
NRC = L // 512           # 8 row chunks
KC = D // 128            # 4 contraction chunks


def _build_masks(nc, pool):
    """Six [128, 512] bf16 {0,1} mask tiles for S^T tiles [c=col, rr=row].

    d0..d3: diagonal-range masks, allowed iff 128*d + c <= rr.
    j0r0:   col block 0, row chunk 0: (c <= rr) and (c >= 1 or rr == 0).
    j0:     col block 0, row chunk > 0: c >= 1.
    """
    masks = {}
    for d in range(4):
        m = pool.tile([128, 512], BF16, name=f"mask_d{d}")
        nc.gpsimd.memset(m, 1.0)
        # value = base + cm*partition + step*free ; keep where >= 0 else fill
        nc.gpsimd.affine_select(
            out=m, in_=m, compare_op=mybir.AluOpType.is_ge, fill=0.0,
            base=-128 * d, channel_multiplier=-1, pattern=[[1, 512]],
        )
        masks[f"d{d}"] = m
    j0r0 = pool.tile([128, 512], BF16, name="mask_j0r0")
    nc.gpsimd.memset(j0r0, 1.0)
    nc.gpsimd.affine_select(
        out=j0r0, in_=j0r0, compare_op=mybir.AluOpType.is_ge, fill=0.0,
        base=0, channel_multiplier=-1, pattern=[[1, 512]],
    )
    nc.gpsimd.memset(j0r0[0:1, 1:512], 0.0)   # row 0 of cols: col0 only for row 0
    masks["j0r0"] = j0r0
    j0 = pool.tile([128, 512], BF16, name="mask_j0")
    nc.gpsimd.memset(j0, 1.0)
    nc.gpsimd.memset(j0[0:1, :], 0.0)          # col 0 masked for all rows >= 1
    masks["j0"] = j0
    return masks


def _mask_for(masks, r, j):
    if j == 0:
        return masks["j0r0"] if r == 0 else masks["j0"]
    d = j - 4 * r
    if 0 <= d <= 3:
        return masks[f"d{d}"]
    return None


def build_kernel(dbg_stage="full"):
    nc = bacc.Bacc(trn_type="TRN2", target_bir_lowering=False)

    x_d = nc.declare_dram_parameter("x", [L, D], F32, isOutput=False)
    wq_d = nc.declare_dram_parameter("wq", [D, CD], F32, isOutput=False)
    wk_d = nc.declare_dram_parameter("wk", [D, CD], F32, isOutput=False)
    wv_d = nc.declare_dram_parameter("wv", [D, CD], F32, isOutput=False)
    wo_d = nc.declare_dram_parameter("wo", [CD, D], F32, isOutput=False)
    bq_d = nc.declare_dram_parameter("bq", [CD], F32, isOutput=False)
    bk_d = nc.declare_dram_parameter("bk", [CD], F32, isOutput=False)
    bv_d = nc.declare_dram_parameter("bv", [CD], F32, isOutput=False)
    out_d = nc.declare_dram_parameter("out", [L, D], F32, isOutput=True)

    with tile.TileContext(nc) as tc:
        with (
            tc.tile_pool(name="const", bufs=1) as const,
            tc.tile_pool(name="stage", bufs=3) as stage,
            tc.tile_pool(name="work", bufs=4) as work,
        ):
            # ---- constants / static tensors
            ident = const.tile([128, 128], BF16, name="ident")
            make_identity(nc, ident)
            masks = _build_masks(nc, const)
            ones_k1 = const.tile([1, CD], BF16, name="ones_k1")
            nc.vector.memset(ones_k1, 1.0)

            xT = const.tile([128, KC, L], BF16, name="xT")        # [D-chunk, d, L]
            qT = const.tile([128, L], BF16, name="qT")            # 2 heads stacked
            kT = const.tile([128, L], BF16, name="kT")
            v0 = const.tile([128, RB, 65], BF16, name="v0")       # v_aug per col block
            v1 = const.tile([128, RB, 65], BF16, name="v1")
            nc.vector.memset(v0[:, :, 64:65], 1.0)
            nc.vector.memset(v1[:, :, 64:65], 1.0)
            O_all = const.tile([64, HPC, L], BF16, name="O_all")  # normalized attn out^T

            # ---- weights: load f32, cast to bf16 in matmul layouts
            wq_f = const.tile([128, KC, CD], F32, name="wq_f")
            nc.sync.dma_start(wq_f, wq_d.rearrange("(o p) c -> p o c", p=128))
            wq_b = const.tile([128, KC, CD], BF16, name="wq_b")
            nc.vector.tensor_copy(wq_b, wq_f)

            wk_f = const.tile([128, KC, CD], F32, name="wk_f")
            nc.sync.dma_start(wk_f, wk_d.rearrange("(o p) c -> p o c", p=128))
            wk_b = const.tile([128, KC, CD], BF16, name="wk_b")
            nc.vector.tensor_copy(wk_b, wk_f)

            wv_f = const.tile([128, KC, CD], F32, name="wv_f")
            nc.sync.dma_start(wv_f, wv_d.rearrange("(o p) c -> p o c", p=128))
            wv_b = const.tile([128, KC, CD], BF16, name="wv_b")
            nc.vector.tensor_copy(wv_b, wv_f)

            # wo: [128, 512] -> [64 rows, 2 heads, 512] (head on free dim, lane aligned)
            wo_f = const.tile([64, HPC, D], F32, name="wo_f")
            nc.sync.dma_start(wo_f, wo_d.rearrange("(h r) n -> r h n", h=HPC))
            wo_b = const.tile([64, HPC, D], BF16, name="wo_b")
            nc.vector.tensor_copy(wo_b, wo_f)

            bq_s = const.tile([CD, 1], F32, name="bq_s")
            nc.sync.dma_start(bq_s, bq_d.rearrange("(p o) -> p o", o=1))
            bk_s = const.tile([CD, 1], F32, name="bk_s")
            nc.sync.dma_start(bk_s, bk_d.rearrange("(p o) -> p o", o=1))
            bv_f = const.tile([1, CD], F32, name="bv_f")
            nc.sync.dma_start(bv_f, bv_d.rearrange("(o c) -> o c", o=1))
            bv_b = const.tile([1, CD], BF16, name="bv_b")
            nc.vector.tensor_copy(bv_b, bv_f)

            # debug: dump a bf16 sbuf AP (viewed [p, n*512]) into out (cast f32)
            _dump_col = [0]

            def _dump(src_bf):
                ov = out_d.rearrange("(o p) c -> p o c", p=128)
                src3 = src_bf.rearrange("p (n c) -> p n c", c=512)
                p = src3.shape[0]
                for i in range(src3.shape[1]):
                    t = stage.tile([128, 512], F32, tag="dumpt")
                    nc.vector.tensor_copy(t[:p, :], src3[:, i, :])
                    nc.sync.dma_start(ov[:p, _dump_col[0], :], t[:p, :])
                    _dump_col[0] += 1

            # ---- phase A: xT via PE transpose; phase B: qkv projections
            with tc.tile_pool(name="psAB", bufs=2, space="PSUM") as psAB:
                for rb in range(RB):
                    xf = stage.tile([128, D], F32, tag="xf")
                    nc.sync.dma_start(xf, x_d[rb * 128:(rb + 1) * 128, :])
                    xb = stage.tile([128, D], BF16, tag="xb")
                    nc.vector.tensor_copy(xb, xf)
                    for d in range(KC):
                        pt = psAB.tile([128, 128], BF16, tag="pt", bufs=3)
                        nc.tensor.transpose(pt, xb[:, d * 128:(d + 1) * 128], ident)
                        nc.any.tensor_copy(xT[:, d, rb * 128:(rb + 1) * 128], pt)

                if dbg_stage == "xt":
                    _dump(xT.rearrange("p o c -> p (o c)"))

                for nb in range(L // 512) if dbg_stage != "xt" else []:
                    ns = slice(nb * 512, (nb + 1) * 512)
                    for wt, bt, dstT in ((wq_b, bq_s, qT), (wk_b, bk_s, kT)):
                        pq = psAB.tile([128, 512], F32, tag="pq", bufs=2)
                        for d in range(KC):
                            nc.tensor.matmul(
                                pq, lhsT=wt[:, d, :], rhs=xT[:, d, ns],
                                start=(d == 0), stop=(d == KC - 1),
                            )
                        nc.vector.tensor_scalar_add(dstT[:, ns], pq, bt)

                for rb in range(RB) if dbg_stage != "xt" else []:
                    rs = slice(rb * 128, (rb + 1) * 128)
                    pv = psAB.tile([128, 512], F32, tag="pq", bufs=2)
                    for d in range(KC):
                        nc.tensor.matmul(
                            pv[:, :CD], lhsT=xT[:, d, rs], rhs=wv_b[:, d, :],
                            start=(d == 0), stop=False,
                        )
                    nc.tensor.matmul(
                        pv[:, :CD], lhsT=ones_k1, rhs=bv_b, start=False, stop=True,
                    )
                    nc.any.tensor_copy(v0[:, rb, 0:64], pv[:, 0:64])
                    nc.any.tensor_copy(v1[:, rb, 0:64], pv[:, 64:128])

                if dbg_stage == "qkv":
                    _dump(qT)
                    _dump(kT)
                    _dump(v0.rearrange("p o c -> p (o c)")[:, :2048])

            # ---- phase C: attention per head; phase D: output projection
            with (
                tc.tile_pool(name="psC", bufs=1, space="PSUM") as psC,
                tc.tile_pool(name="psD", bufs=3, space="PSUM") as psD,
                tc.tile_pool(name="dramp", bufs=3, space="DRAM") as dramp,
            ):
                for h in range(HPC) if dbg_stage in ("attn", "full") else []:
                    hs = slice(h * 64, (h + 1) * 64)
                    vh = v0 if h == 0 else v1
                    for r in range(NRC):
                        rs = slice(r * 512, (r + 1) * 512)
                        po_t = psC.tile([65, 512], F32, tag="po", bufs=2)
                        njb = 4 * r + 4
                        for j in range(njb):
                            ss = psC.tile([128, 512], F32, tag="ss", bufs=3)
                            nc.tensor.matmul(
                                ss, lhsT=kT[hs, j * 128:(j + 1) * 128],
                                rhs=qT[hs, rs], start=True, stop=True,
                            )
                            p_sb = work.tile([128, 512], BF16, tag="p_sb")
                            nc.scalar.activation(
                                p_sb, ss, mybir.ActivationFunctionType.Exp,
                                scale=SCALE,
                            )
                            m = _mask_for(masks, r, j)
                            if m is not None:
                                nc.vector.tensor_mul(out=p_sb, in0=p_sb, in1=m)
                            nc.tensor.matmul(
                                po_t, lhsT=vh[:, j, :], rhs=p_sb,
                                start=(j == 0), stop=(j == njb - 1),
                            )
                        rr_t = work.tile([65, 512], F32, tag="rr")
                        nc.vector.reciprocal(rr_t[64:65, :], po_t[64:65, :])
                        # broadcast partition 64 -> 0..63 via DRAM bounce
                        # (gpsimd partition_broadcast crashes the exec unit on HW)
                        scr = dramp.tile([1, 512], F32, tag="scr")
                        nc.sync.dma_start(out=scr[0:1, :], in_=rr_t[64:65, :])
                        s = scr[0:1, :]
                        src_b = bass.AP(
                            tensor=s.tensor, offset=s.offset,
                            ap=[[0, 64]] + [list(p) for p in s.ap[1:]],
                        )
                        nc.sync.dma_start(out=rr_t[0:64, :], in_=src_b)
                        nc.vector.tensor_tensor(
                            O_all[:, h, rs], po_t[0:64, :], rr_t[0:64, :],
                            mybir.AluOpType.mult,
                        )

                if dbg_stage == "attn":
                    _dump(O_all.rearrange("p h c -> p (h c)"))

                for rb in range(RB) if dbg_stage == "full" else []:
                    rs = slice(rb * 128, (rb + 1) * 128)
                    pod = psD.tile([128, 512], F32, tag="pod", bufs=3)
                    for h in range(HPC):
                        nc.tensor.matmul(
                            pod, lhsT=O_all[:, h, rs], rhs=wo_b[:, h, :],
                            start=(h == 0), stop=(h == HPC - 1),
                        )
                    ot = stage.tile([128, D], F32, tag="ot")
                    nc.any.tensor_copy(ot, pod)
                    nc.sync.dma_start(out_d[rs, :], ot)

    nc.finalize()
    return nc


def _shard_inputs(x, w_in, b_in, w_out):
    """Per-core input maps: core c -> batch c//4, heads pair c%4."""
    in_maps = []
    for c in range(NCORES):
        b = c // 4
        hp = c % 4
        cs = slice(hp * CD, hp * CD + CD)
        in_maps.append({
            "x": np.ascontiguousarray(x[b]),
            "wq": np.ascontiguousarray(w_in[:, 0:D][:, cs]),
            "wk": np.ascontiguousarray(w_in[:, D:2 * D][:, cs]),
            "wv": np.ascontiguousarray(w_in[:, 2 * D:3 * D][:, cs]),
            "wo": np.ascontiguousarray(w_out[cs, :]),
            "bq": np.ascontiguousarray(b_in[0:D][cs]),
            "bk": np.ascontiguousarray(b_in[D:2 * D][cs]),
            "bv": np.ascontiguousarray(b_in[2 * D:3 * D][cs]),
        })
    return in_maps


_NC_CACHE = None


def _get_nc():
    global _NC_CACHE
    if _NC_CACHE is None:
        _NC_CACHE = build_kernel()
    return _NC_CACHE


def run(x, w_in, b_in, w_out, b_out, trace=False, **spmd_kwargs):
    x = np.asarray(x, dtype=np.float32)
    w_in = np.asarray(w_in, dtype=np.float32)
    b_in = np.asarray(b_in, dtype=np.float32)
    w_out = np.asarray(w_out, dtype=np.float32)
    b_out = np.asarray(b_out, dtype=np.float32)

    nc = _get_nc()
    in_maps = _shard_inputs(x, w_in, b_in, w_out)
    res = run_bass_kernel_spmd(
        nc, in_maps, core_ids=list(range(NCORES)), trace=trace, **spmd_kwargs
    )
    out = np.zeros((B, L, D), dtype=np.float32)
    for c in range(NCORES):
        out[c // 4] += res.results[c]["out"]
    out += b_out[None, None, :]
    return out, res


def kernel(x, w_in, b_in, w_out, b_out):
    out, _ = run(x, w_in, b_in, w_out, b_out, trace=False)
    return out

